# revision 17
# baseline (speedup 1.0000x reference)
"""AIMNet-style embedding kernel for 8 Trainium2 NeuronCores.

Data-parallel over the molecule batch B=8 (one molecule per core).
Host-side prep does layout transforms only (transpose ga/gr so the
contraction axis is on partitions, bf16 casts, small broadcast tables);
all FLOPs run on device.

Per-core device pipeline (molecule b):
  1. pair build:  X^T[128, 8128]  (one tensor_scalar per anchor atom i,
     split across DVE and GpSimd)
  2. combine MLP: C1^T = cw1^T @ X^T -> gelu -> G1^T ;  FP = G1^T chunks @ cw2
  3. grv:  afv^T @ grT slices  (per radial shift r)     -> Z^T k-tiles 0..15
  4. gav:  FP_k^T @ gaT k-tiles (64 accumulating steps into psum[32, 2048])
  5. embed MLP: accumulate psum[n, 512] over Z^T k-tiles (Z as stationary,
     ew1 as moving; eb1 folded in as a rank-1 matmul), gelu, PE-transpose
     A1 -> A1^T, then ew2^T @ A1^T -> AEF^T (+eb2) -> out

Stages 1/2/4 are emitted interleaved so the PE's in-order stream starts
consuming gaT tiles as soon as the first FP tiles exist, and the zt-part
of the embed accumulation runs mid-stream; only the 4 gav k-tiles of the
embed remain after the last gaT tile.
"""

import numpy as np
import ml_dtypes

import concourse.bass as bass
import concourse.mybir as mybir
import concourse.tile as tile
from concourse import bacc
from concourse.bass_utils import run_bass_kernel_spmd
from concourse.masks import make_identity

BF16NP = ml_dtypes.bfloat16
F32 = mybir.dt.float32
BF = mybir.dt.bfloat16

B, N, A = 8, 128, 64
Rr, Ra = 32, 16
P = N * (N - 1) // 2          # 8128
D = 32                        # d_pair
H, E = 512, 256
M2 = N * Ra                   # 2048 = gav output dim (r'-major: m = r'*128 + n)
G2 = N * Rr                   # 4096 = grT cols (r-major: r*128 + n)
NKT = (P + N - 1) // N        # 64 pair k-tiles (63 full + one of 64)

GELU = mybir.ActivationFunctionType.Gelu_apprx_tanh
IDENT = mybir.ActivationFunctionType.Identity
MULT = mybir.AluOpType.mult
ADD = mybir.AluOpType.add

_CACHE: dict = {}


def _build_nc():
    nc = bacc.Bacc("TRN2", target_bir_lowering=False)

    gaT = nc.dram_tensor("gaT", [P, M2], BF, kind="ExternalInput")
    grT = nc.dram_tensor("grT", [N, G2], BF, kind="ExternalInput")
    afv = nc.dram_tensor("afv", [N, A], BF, kind="ExternalInput")
    afv2 = nc.dram_tensor("afv2", [2 * A, N], F32, kind="ExternalInput")
    s1 = nc.dram_tensor("s1", [2 * A, N], F32, kind="ExternalInput")
    s2 = nc.dram_tensor("s2", [2 * A, N], F32, kind="ExternalInput")
    cw1 = nc.dram_tensor("cw1", [2 * A, 2 * A], BF, kind="ExternalInput")
    cw2 = nc.dram_tensor("cw2", [2 * A, D], BF, kind="ExternalInput")
    cb1 = nc.dram_tensor("cb1", [2 * A, 1], F32, kind="ExternalInput")
    cb2b = nc.dram_tensor("cb2b", [2 * A, D], F32, kind="ExternalInput")
    ew1 = nc.dram_tensor("ew1", [2560, H], BF, kind="ExternalInput")
    eb1r = nc.dram_tensor("eb1r", [1, H], BF, kind="ExternalInput")
    ew2 = nc.dram_tensor("ew2", [H, E], BF, kind="ExternalInput")
    eb2 = nc.dram_tensor("eb2", [E], F32, kind="ExternalInput")
    out = nc.dram_tensor("out", [E, N], F32, kind="ExternalOutput")

    with tile.TileContext(nc) as tc:
        with (
            tc.tile_pool(name="const", bufs=1) as cp,
            tc.tile_pool(name="big", bufs=1) as bp,
            tc.tile_pool(name="ga", bufs=10) as gap,
        ):
            # ---- constants / small tensors ----
            cw1s = cp.tile([128, 128], BF)
            nc.sync.dma_start(out=cw1s, in_=cw1[:])
            cw2s = cp.tile([128, D], BF)
            nc.sync.dma_start(out=cw2s, in_=cw2[:])
            cb1s = cp.tile([128, 1], F32)
            nc.sync.dma_start(out=cb1s, in_=cb1[:])
            cb2bs = cp.tile([128, D], F32)
            nc.sync.dma_start(out=cb2bs, in_=cb2b[:])
            afvs = cp.tile([128, A], BF)
            nc.sync.dma_start(out=afvs, in_=afv[:])
            afv2s = cp.tile([128, N], F32)
            nc.sync.dma_start(out=afv2s, in_=afv2[:])
            s1s = cp.tile([128, N], F32)
            nc.sync.dma_start(out=s1s, in_=s1[:])
            s2s = cp.tile([128, N], F32)
            nc.sync.dma_start(out=s2s, in_=s2[:])
            eb1rs = cp.tile([1, H], BF)
            nc.sync.dma_start(out=eb1rs, in_=eb1r[:])
            eb2s = cp.tile([128, 2], F32)
            nc.sync.dma_start(out=eb2s, in_=eb2[:].rearrange("(c p) -> p c", p=128))
            grts = cp.tile([128, G2], BF)
            ew1s = cp.tile([128, 20, H], BF)
            ew2s = cp.tile([128, 4, E], BF)

            ones1 = cp.tile([1, N], BF)
            nc.vector.memset(ones1, 1.0)
            ident = cp.tile([128, 128], BF)
            make_identity(nc, ident)

            # ---- persistent intermediates ----
            xt = bp.tile([128, P], BF)           # X^T  (pair features)
            g1t = bp.tile([128, P], BF)          # gelu(C1)^T
            fps = bp.tile([128, NKT * D], BF)    # FP, k-tile q at cols [q*32, q*32+32)
            zt = bp.tile([128, 16 * N], BF)      # Z^T grv part, k-tile kt at cols kt*128
            ztg = bp.tile([128, 4, N], BF)       # Z^T gav part, packed 4 pieces/k-tile
            a1 = bp.tile([128, H], BF)           # A1 [n, h]
            a1t = bp.tile([128, 4, N], BF)       # A1^T, h-chunk ht at [:, ht, :]
            aeft = bp.tile([128, 2, N], F32)     # AEF^T chunks

            # ---- stages 1+2+4 interleaved ----
            # Pair block i (i=0..126) covers pairs (i, j) j=i+1..127 (width
            # 127-i).  Emission is pipelined per 512-pair C1 chunk, and the
            # gav matmuls for group g (FP chunks 4g..4g+3 = DMA pairs 2g,
            # 2g+1) are emitted right after the FP tiles they need, so the
            # PE's in-order stream never parks gav work behind the whole
            # pair-build chain.
            offs = np.concatenate([[0], np.cumsum(N - 1 - np.arange(N - 1))])
            next_blk = 0

            def emit_pair_blocks_until(cov):
                nonlocal next_blk
                while next_blk < N - 1 and offs[next_blk] < cov:
                    i = next_blk
                    eng = nc.gpsimd if i % 3 == 2 else nc.vector
                    eng.tensor_scalar(
                        out=xt[:, offs[i]:offs[i + 1]],
                        in0=afv2s[:, i + 1:N],
                        scalar1=s1s[:, i:i + 1],
                        scalar2=s2s[:, i:i + 1],
                        op0=MULT,
                        op1=ADD,
                    )
                    next_blk += 1

            # PSUM pool lifetimes (explicit, LIFO):
            #   psGav (4 banks) spans the whole stream;
            #   psA (c1/fp shared tag, 2 banks) spans stages 1+2;
            #   psGrv (2 banks) closes after grv, freeing room for ps1 (1).
            psGav_cm = tc.tile_pool(name="psGav", bufs=1, space="PSUM")
            psGav = psGav_cm.__enter__()
            psA_cm = tc.tile_pool(name="psA", bufs=2, space="PSUM")
            psA = psA_cm.__enter__()
            psGrv_cm = tc.tile_pool(name="psGrv", bufs=1, space="PSUM")
            psGrv = psGrv_cm.__enter__()

            psg = psGav.tile([32, M2], F32)

            def emit_c1(pc):
                w = min(512, P - pc * 512)
                ps = psA.tile([128, 512], F32, tag="c1")
                nc.tensor.matmul(
                    ps[:, 0:w], cw1s[:, :], xt[:, pc * 512:pc * 512 + w],
                    start=True, stop=True,
                )
                nc.scalar.activation(
                    g1t[:, pc * 512:pc * 512 + w], ps[:, 0:w], GELU,
                    bias=cb1s[:, 0:1], scale=1.0,
                )

            def emit_fp(q):
                kw = min(128, P - q * 128)
                ps = psA.tile([128, 512], F32, tag="c1")
                nc.tensor.matmul(
                    ps[0:kw, 0:D], g1t[:, q * 128:q * 128 + kw], cw2s[:, :],
                    start=True, stop=True,
                )
                nc.vector.tensor_tensor(
                    out=fps[0:kw, q * D:(q + 1) * D],
                    in0=ps[0:kw, 0:D],
                    in1=cb2bs[0:kw, :],
                    op=ADD,
                )

            def emit_gav_pair(dm):
                # alternate the two HWDGE rings (SP / ACT) so transfers
                # overlap across DMA boundaries
                dmae = nc.sync if dm % 2 == 0 else nc.scalar
                ga_t = gap.tile([128, 2, M2], BF, tag="ga")
                if dm < 31:
                    dmae.dma_start(
                        out=ga_t,
                        in_=gaT[dm * 256:(dm + 1) * 256, :].rearrange(
                            "(two p) m -> p two m", two=2
                        ),
                    )
                else:
                    dmae.dma_start(out=ga_t[:, 0, :], in_=gaT[7936:8064, :])
                    dmae.dma_start(out=ga_t[0:64, 1, :], in_=gaT[8064:8128, :])
                for half in range(2):
                    kt = dm * 2 + half
                    kw = 64 if kt == NKT - 1 else 128
                    for mc in range(4):
                        nc.tensor.matmul(
                            psg[:, mc * 512:(mc + 1) * 512],
                            fps[0:kw, kt * D:(kt + 1) * D],
                            ga_t[0:kw, half, mc * 512:(mc + 1) * 512],
                            start=(kt == 0),
                            stop=(kt == NKT - 1),
                        )

            def emit_group(g):
                for q in range(4 * g, 4 * g + 4):
                    emit_fp(q)
                emit_gav_pair(2 * g)
                emit_gav_pair(2 * g + 1)

            ps1 = None
            psE1_cm = None
            for pc in range(16):
                emit_pair_blocks_until((pc + 1) * 512)
                emit_c1(pc)
                if pc == 1:
                    # bulk weight loads via SWDGE (separate DGE; the two
                    # HWDGE rings stay dedicated to the gaT stream)
                    nc.gpsimd.dma_start(out=grts, in_=grT[:])
                    nc.gpsimd.dma_start(
                        out=ew1s, in_=ew1[:].rearrange("(t p) h -> p t h", p=128)
                    )
                    nc.gpsimd.dma_start(
                        out=ew2s, in_=ew2[:].rearrange("(t p) e -> p t e", p=128)
                    )
                if pc >= 2:
                    emit_group(pc - 2)
                if pc == 10:
                    # grv in two psum rounds; r -> (kt=r//2, half=r%2);
                    # grT is r-major so the moving operand is contiguous
                    for rnd in range(2):
                        ps_grv = psGrv.tile([128, 8, N], F32, tag="grv")
                        for rr in range(16):
                            r = rnd * 16 + rr
                            base = (r % 2) * 64
                            nc.tensor.matmul(
                                ps_grv[base:base + 64, rr // 2, :],
                                afvs[:, :],
                                grts[:, r * N:(r + 1) * N],
                                start=True,
                                stop=True,
                                tile_position=(0, base),
                            )
                        nc.vector.tensor_copy(
                            zt[:, rnd * 8 * N:(rnd + 1) * 8 * N], ps_grv[:, :, :]
                        )
                    psGrv_cm.__exit__(None, None, None)
                    psE1_cm = tc.tile_pool(name="psE1", bufs=1, space="PSUM")
                    psE1 = psE1_cm.__enter__()
                    ps1 = psE1.tile([128, H], F32)
                if pc == 14:
                    # A1[n, h] accumulation: rank-1 eb1 + 16 grv k-tiles now,
                    # 4 gav k-tiles at the very end
                    nc.tensor.matmul(ps1, ones1, eb1rs, start=True, stop=False)
                    for kt in range(16):
                        nc.tensor.matmul(
                            ps1,
                            zt[:, kt * N:(kt + 1) * N],
                            ew1s[:, kt, :],
                            start=False,
                            stop=False,
                        )
            for g in (14, 15):
                emit_group(g)

            # pack the 16 [32, n] gav pieces into 4 full 128-partition
            # k-tiles (piece r'=4t+q -> partitions q*32.., k-tile t), split
            # across DVE and ACT; gaT is r'-major so reads are contiguous
            for rp in range(Ra):
                t, q = rp // 4, rp % 4
                dst = ztg[q * 32:(q + 1) * 32, t, :]
                src = psg[:, rp * N:(rp + 1) * N]
                if rp % 3 == 2:
                    nc.scalar.activation(dst, src, IDENT)
                else:
                    nc.vector.tensor_copy(dst, src)
            for t in range(4):
                nc.tensor.matmul(
                    ps1,
                    ztg[:, t, :],
                    ew1s[:, 16 + t, :],
                    start=False,
                    stop=(t == 3),
                )
            nc.scalar.activation(a1, ps1, GELU, bias=0.0, scale=1.0)

            psE1_cm.__exit__(None, None, None)
            psA_cm.__exit__(None, None, None)
            psGav_cm.__exit__(None, None, None)

            # ---- stage 5 tail: transpose A1, final projection ----
            with tc.tile_pool(name="psE2", bufs=2, space="PSUM") as psE2:
                for ht in range(4):
                    tr = psE2.tile([128, N], BF, tag="tr")
                    nc.tensor.transpose(tr, a1[:, ht * 128:(ht + 1) * 128], ident)
                    nc.vector.tensor_copy(a1t[:, ht, :], tr)
                for ec in range(2):
                    ps2 = psE2.tile([128, N], F32, tag="aef")
                    for ht in range(4):
                        nc.tensor.matmul(
                            ps2,
                            ew2s[:, ht, ec * 128:(ec + 1) * 128],
                            a1t[:, ht, :],
                            start=(ht == 0),
                            stop=(ht == 3),
                        )
                    nc.scalar.activation(
                        aeft[:, ec, :], ps2, IDENT, bias=eb2s[:, ec:ec + 1], scale=1.0,
                    )
                nc.sync.dma_start(
                    out=out[:].rearrange("(c e) n -> e c n", c=2), in_=aeft
                )

    nc.compile()
    return nc


def _get_nc():
    if "nc" not in _CACHE:
        _CACHE["nc"] = _build_nc()
    return _CACHE["nc"]


def _prep_in_maps(gr, ga, afv, cw1, cb1, cw2, cb2, ew1, eb1, ew2, eb2):
    gr = np.asarray(gr, np.float32)
    ga = np.asarray(ga, np.float32)
    afv = np.asarray(afv, np.float32)
    cw1 = np.asarray(cw1, np.float32)
    cb1 = np.asarray(cb1, np.float32)
    cw2 = np.asarray(cw2, np.float32)
    cb2 = np.asarray(cb2, np.float32)
    ew1 = np.asarray(ew1, np.float32)
    eb1 = np.asarray(eb1, np.float32)
    ew2 = np.asarray(ew2, np.float32)
    eb2 = np.asarray(eb2, np.float32)

    shared = {
        "cw1": np.ascontiguousarray(cw1.astype(BF16NP)),
        "cw2": np.ascontiguousarray(cw2.astype(BF16NP)),
        "cb1": np.ascontiguousarray(cb1.reshape(2 * A, 1)),
        "cb2b": np.ascontiguousarray(np.broadcast_to(cb2, (2 * A, D))),
        "ew1": np.ascontiguousarray(ew1.astype(BF16NP)),
        "eb1r": np.ascontiguousarray(eb1.reshape(1, H).astype(BF16NP)),
        "ew2": np.ascontiguousarray(ew2.astype(BF16NP)),
        "eb2": np.ascontiguousarray(eb2),
    }
    in_maps = []
    ones64 = np.ones((A, N), np.float32)
    zeros64 = np.zeros((A, N), np.float32)
    for b in range(B):
        afvT = np.ascontiguousarray(afv[b].T)  # [64, 128]
        m = dict(shared)
        # gaT: [P, (r', n)] r'-major columns
        m["gaT"] = np.ascontiguousarray(
            ga[b].transpose(1, 0, 2).reshape(M2, P).T.astype(BF16NP)
        )
        # grT: [m, (r, n)] r-major so per-r rhs slices are contiguous
        m["grT"] = np.ascontiguousarray(
            gr[b].transpose(2, 1, 0).reshape(N, G2).astype(BF16NP)
        )
        m["afv"] = np.ascontiguousarray(afv[b].astype(BF16NP))
        m["afv2"] = np.ascontiguousarray(np.concatenate([afvT, afvT], axis=0))
        m["s1"] = np.ascontiguousarray(np.concatenate([ones64, afvT], axis=0))
        m["s2"] = np.ascontiguousarray(np.concatenate([afvT, zeros64], axis=0))
        in_maps.append(m)
    return in_maps


def run(inputs: dict, trace: bool = False):
    """Returns ((aef, afv), exec_time_ns_or_None)."""
    nc = _get_nc()
    in_maps = _prep_in_maps(**inputs)
    res = run_bass_kernel_spmd(nc, in_maps, core_ids=list(range(B)), trace=trace)
    aef = np.stack(
        [np.ascontiguousarray(res.results[b]["out"].T) for b in range(B)], axis=0
    )
    afv = np.asarray(inputs["afv"], np.float32)
    return (aef, afv), res.exec_time_ns


def kernel(**inputs) -> np.ndarray:
    (aef, afv), _ = run(inputs, trace=False)
    return aef, afv


# revision 18
# speedup vs baseline: 1.2557x; 1.2557x over previous
"""AIMNet-style embedding kernel for 8 Trainium2 NeuronCores.

Data-parallel over the molecule batch B=8 (one molecule per core).
Host-side prep does layout transforms only (transpose ga/gr so the
contraction axis is on partitions, bf16 casts, small broadcast tables);
all FLOPs run on device.

Per-core device pipeline (molecule b):
  1. pair build:  X^T[128, 8128]  (one tensor_scalar per anchor atom i,
     split across DVE and GpSimd)
  2. combine MLP: C1^T = cw1^T @ X^T -> gelu -> G1^T ;  FP = G1^T chunks @ cw2
  3. grv:  afv^T @ grT slices  (per radial shift r)     -> Z^T k-tiles 0..15
  4. gav:  FP_k^T @ gaT k-tiles (64 accumulating steps into psum[32, 2048])
  5. embed MLP: accumulate psum[n, 512] over Z^T k-tiles (Z as stationary,
     ew1 as moving; eb1 folded in as a rank-1 matmul), gelu, PE-transpose
     A1 -> A1^T, then ew2^T @ A1^T -> AEF^T (+eb2) -> out

Stages 1/2/4 are emitted interleaved so the PE's in-order stream starts
consuming gaT tiles as soon as the first FP tiles exist, and the zt-part
of the embed accumulation runs mid-stream; only the 4 gav k-tiles of the
embed remain after the last gaT tile.
"""

import numpy as np
import ml_dtypes

import concourse.bass as bass
import concourse.mybir as mybir
import concourse.tile as tile
from concourse import bacc
from concourse.bass_utils import run_bass_kernel_spmd
from concourse.masks import make_identity

BF16NP = ml_dtypes.bfloat16
F32 = mybir.dt.float32
BF = mybir.dt.bfloat16

B, N, A = 8, 128, 64
Rr, Ra = 32, 16
P = N * (N - 1) // 2          # 8128
D = 32                        # d_pair
H, E = 512, 256
M2 = N * Ra                   # 2048 = gav output dim (r'-major: m = r'*128 + n)
G2 = N * Rr                   # 4096 = grT cols (r-major: r*128 + n)
NKT = (P + N - 1) // N        # 64 pair k-tiles (63 full + one of 64)

GELU = mybir.ActivationFunctionType.Gelu_apprx_tanh
IDENT = mybir.ActivationFunctionType.Identity
MULT = mybir.AluOpType.mult
ADD = mybir.AluOpType.add

_CACHE: dict = {}


def _build_nc():
    nc = bacc.Bacc("TRN2", target_bir_lowering=False)

    gaT = nc.dram_tensor("gaT", [P, M2], BF, kind="ExternalInput")
    grT = nc.dram_tensor("grT", [N, G2], BF, kind="ExternalInput")
    # packF cols: cb1 0:1 | cb2b 1:33 | afv2 33:161 | s1 161:289 | s2 289:417 | eb2t 417:419
    packF = nc.dram_tensor("packF", [128, 419], F32, kind="ExternalInput")
    # packB cols: cw1 0:128 | cw2 128:160 | afv 160:224
    packB = nc.dram_tensor("packB", [128, 224], BF, kind="ExternalInput")
    eb1r = nc.dram_tensor("eb1r", [1, H], BF, kind="ExternalInput")
    ew1t = nc.dram_tensor("ew1t", [128, 20 * H], BF, kind="ExternalInput")
    ew2t = nc.dram_tensor("ew2t", [128, 4 * E], BF, kind="ExternalInput")
    out = nc.dram_tensor("out", [E, N], F32, kind="ExternalOutput")

    with tile.TileContext(nc) as tc:
        with (
            tc.tile_pool(name="const", bufs=1) as cp,
            tc.tile_pool(name="big", bufs=1) as bp,
            tc.tile_pool(name="ga", bufs=10) as gap,
        ):
            # ---- constants (host-packed: 3 small DMAs) ----
            packFs = cp.tile([128, 419], F32)
            nc.sync.dma_start(out=packFs, in_=packF[:])
            packBs = cp.tile([128, 224], BF)
            nc.sync.dma_start(out=packBs, in_=packB[:])
            eb1rs = cp.tile([1, H], BF)
            nc.sync.dma_start(out=eb1rs, in_=eb1r[:])
            cb1s = packFs[:, 0:1]
            cb2bs = packFs[:, 1:33]
            afv2s = packFs[:, 33:161]
            s1s = packFs[:, 161:289]
            s2s = packFs[:, 289:417]
            eb2s = packFs[:, 417:419]
            cw1s = packBs[:, 0:128]
            cw2s = packBs[:, 128:160]
            afvs = packBs[:, 160:224]
            # bulk weights on the scalar HWDGE ring, ahead of the odd gaT
            # tiles (host-pretiled: contiguous per-partition runs)
            grts = cp.tile([128, G2], BF)
            nc.scalar.dma_start(out=grts, in_=grT[:])
            ew2s = cp.tile([128, 4, E], BF)
            nc.scalar.dma_start(
                out=ew2s, in_=ew2t[:].rearrange("p (t e) -> p t e", t=4)
            )
            ew1s = cp.tile([128, 20, H], BF)
            nc.scalar.dma_start(
                out=ew1s, in_=ew1t[:].rearrange("p (t h) -> p t h", t=20)
            )

            ones1 = cp.tile([1, N], BF)
            nc.vector.memset(ones1, 1.0)
            ident = cp.tile([128, 128], BF)
            make_identity(nc, ident)

            # ---- persistent intermediates ----
            xt = bp.tile([128, P], BF)           # X^T  (pair features)
            g1t = bp.tile([128, P], BF)          # gelu(C1)^T
            fps = bp.tile([128, NKT * D], BF)    # FP, k-tile q at cols [q*32, q*32+32)
            zt = bp.tile([128, 16 * N], BF)      # Z^T grv part, k-tile kt at cols kt*128
            ztg = bp.tile([128, 4, N], BF)       # Z^T gav part, packed 4 pieces/k-tile
            a1 = bp.tile([128, H], BF)           # A1 [n, h]
            a1t = bp.tile([128, 4, N], BF)       # A1^T, h-chunk ht at [:, ht, :]
            aeft = bp.tile([128, 2, N], F32)     # AEF^T chunks

            # ---- stages 1+2+4 interleaved ----
            # Pair block i (i=0..126) covers pairs (i, j) j=i+1..127 (width
            # 127-i).  Emission is pipelined per 512-pair C1 chunk, and the
            # gav matmuls for group g (FP chunks 4g..4g+3 = DMA pairs 2g,
            # 2g+1) are emitted right after the FP tiles they need, so the
            # PE's in-order stream never parks gav work behind the whole
            # pair-build chain.
            offs = np.concatenate([[0], np.cumsum(N - 1 - np.arange(N - 1))])
            next_blk = 0

            def emit_pair_blocks_until(cov):
                nonlocal next_blk
                while next_blk < N - 1 and offs[next_blk] < cov:
                    i = next_blk
                    eng = nc.gpsimd if i % 3 == 2 else nc.vector
                    eng.tensor_scalar(
                        out=xt[:, offs[i]:offs[i + 1]],
                        in0=afv2s[:, i + 1:N],
                        scalar1=s1s[:, i:i + 1],
                        scalar2=s2s[:, i:i + 1],
                        op0=MULT,
                        op1=ADD,
                    )
                    next_blk += 1

            # PSUM pool lifetimes (explicit, LIFO):
            #   psGav (4 banks) spans the whole stream;
            #   psA (c1/fp shared tag, 2 banks) spans stages 1+2;
            #   psGrv (2 banks) closes after grv, freeing room for ps1 (1).
            psGav_cm = tc.tile_pool(name="psGav", bufs=1, space="PSUM")
            psGav = psGav_cm.__enter__()
            psA_cm = tc.tile_pool(name="psA", bufs=2, space="PSUM")
            psA = psA_cm.__enter__()
            psGrv_cm = tc.tile_pool(name="psGrv", bufs=1, space="PSUM")
            psGrv = psGrv_cm.__enter__()

            psg = psGav.tile([32, M2], F32)

            def emit_c1(pc):
                w = min(512, P - pc * 512)
                ps = psA.tile([128, 512], F32, tag="c1")
                nc.tensor.matmul(
                    ps[:, 0:w], cw1s[:, :], xt[:, pc * 512:pc * 512 + w],
                    start=True, stop=True,
                )
                nc.scalar.activation(
                    g1t[:, pc * 512:pc * 512 + w], ps[:, 0:w], GELU,
                    bias=cb1s[:, 0:1], scale=1.0,
                )

            def emit_fp(q):
                kw = min(128, P - q * 128)
                ps = psA.tile([128, 512], F32, tag="c1")
                nc.tensor.matmul(
                    ps[0:kw, 0:D], g1t[:, q * 128:q * 128 + kw], cw2s[:, :],
                    start=True, stop=True,
                )
                nc.vector.tensor_tensor(
                    out=fps[0:kw, q * D:(q + 1) * D],
                    in0=ps[0:kw, 0:D],
                    in1=cb2bs[0:kw, :],
                    op=ADD,
                )

            def emit_gav_pair(dm):
                # alternate the two HWDGE rings (SP / ACT) so transfers
                # overlap across DMA boundaries
                dmae = nc.sync if dm % 2 == 0 else nc.scalar
                ga_t = gap.tile([128, 2, M2], BF, tag="ga")
                if dm < 31:
                    dmae.dma_start(
                        out=ga_t,
                        in_=gaT[dm * 256:(dm + 1) * 256, :].rearrange(
                            "(two p) m -> p two m", two=2
                        ),
                    )
                else:
                    dmae.dma_start(out=ga_t[:, 0, :], in_=gaT[7936:8064, :])
                    dmae.dma_start(out=ga_t[0:64, 1, :], in_=gaT[8064:8128, :])
                for half in range(2):
                    kt = dm * 2 + half
                    kw = 64 if kt == NKT - 1 else 128
                    for mc in range(4):
                        nc.tensor.matmul(
                            psg[:, mc * 512:(mc + 1) * 512],
                            fps[0:kw, kt * D:(kt + 1) * D],
                            ga_t[0:kw, half, mc * 512:(mc + 1) * 512],
                            start=(kt == 0),
                            stop=(kt == NKT - 1),
                        )

            def emit_group(g):
                for q in range(4 * g, 4 * g + 4):
                    emit_fp(q)
                emit_gav_pair(2 * g)
                emit_gav_pair(2 * g + 1)

            ps1 = None
            psE1_cm = None
            for pc in range(16):
                emit_pair_blocks_until((pc + 1) * 512)
                emit_c1(pc)
                if pc >= 2:
                    emit_group(pc - 2)
                if pc == 10:
                    # grv in two psum rounds; r -> (kt=r//2, half=r%2);
                    # grT is r-major so the moving operand is contiguous
                    for rnd in range(2):
                        ps_grv = psGrv.tile([128, 8, N], F32, tag="grv")
                        for rr in range(16):
                            r = rnd * 16 + rr
                            base = (r % 2) * 64
                            nc.tensor.matmul(
                                ps_grv[base:base + 64, rr // 2, :],
                                afvs[:, :],
                                grts[:, r * N:(r + 1) * N],
                                start=True,
                                stop=True,
                                tile_position=(0, base),
                            )
                        nc.vector.tensor_copy(
                            zt[:, rnd * 8 * N:(rnd + 1) * 8 * N], ps_grv[:, :, :]
                        )
                    psGrv_cm.__exit__(None, None, None)
                    psE1_cm = tc.tile_pool(name="psE1", bufs=1, space="PSUM")
                    psE1 = psE1_cm.__enter__()
                    ps1 = psE1.tile([128, H], F32)
                if pc == 14:
                    # A1[n, h] accumulation: rank-1 eb1 + 16 grv k-tiles now,
                    # 4 gav k-tiles at the very end
                    nc.tensor.matmul(ps1, ones1, eb1rs, start=True, stop=False)
                    for kt in range(16):
                        nc.tensor.matmul(
                            ps1,
                            zt[:, kt * N:(kt + 1) * N],
                            ew1s[:, kt, :],
                            start=False,
                            stop=False,
                        )
            for g in (14, 15):
                emit_group(g)

            # pack the 16 [32, n] gav pieces into 4 full 128-partition
            # k-tiles (piece r'=4t+q -> partitions q*32.., k-tile t), split
            # across DVE and ACT; gaT is r'-major so reads are contiguous
            for rp in range(Ra):
                t, q = rp // 4, rp % 4
                dst = ztg[q * 32:(q + 1) * 32, t, :]
                src = psg[:, rp * N:(rp + 1) * N]
                if rp % 3 == 2:
                    nc.scalar.activation(dst, src, IDENT)
                else:
                    nc.vector.tensor_copy(dst, src)
            for t in range(4):
                nc.tensor.matmul(
                    ps1,
                    ztg[:, t, :],
                    ew1s[:, 16 + t, :],
                    start=False,
                    stop=(t == 3),
                )
            nc.scalar.activation(a1, ps1, GELU, bias=0.0, scale=1.0)

            psE1_cm.__exit__(None, None, None)
            psA_cm.__exit__(None, None, None)
            psGav_cm.__exit__(None, None, None)

            # ---- stage 5 tail: transpose A1, final projection ----
            with tc.tile_pool(name="psE2", bufs=2, space="PSUM") as psE2:
                for ht in range(4):
                    tr = psE2.tile([128, N], BF, tag="tr")
                    nc.tensor.transpose(tr, a1[:, ht * 128:(ht + 1) * 128], ident)
                    nc.vector.tensor_copy(a1t[:, ht, :], tr)
                for ec in range(2):
                    ps2 = psE2.tile([128, N], F32, tag="aef")
                    for ht in range(4):
                        nc.tensor.matmul(
                            ps2,
                            ew2s[:, ht, ec * 128:(ec + 1) * 128],
                            a1t[:, ht, :],
                            start=(ht == 0),
                            stop=(ht == 3),
                        )
                    nc.scalar.activation(
                        aeft[:, ec, :], ps2, IDENT, bias=eb2s[:, ec:ec + 1], scale=1.0,
                    )
                nc.sync.dma_start(
                    out=out[:].rearrange("(c e) n -> e c n", c=2), in_=aeft
                )

    nc.compile()
    return nc


def _get_nc():
    if "nc" not in _CACHE:
        _CACHE["nc"] = _build_nc()
    return _CACHE["nc"]


def _prep_in_maps(gr, ga, afv, cw1, cb1, cw2, cb2, ew1, eb1, ew2, eb2):
    gr = np.asarray(gr, np.float32)
    ga = np.asarray(ga, np.float32)
    afv = np.asarray(afv, np.float32)
    cw1 = np.asarray(cw1, np.float32)
    cb1 = np.asarray(cb1, np.float32)
    cw2 = np.asarray(cw2, np.float32)
    cb2 = np.asarray(cb2, np.float32)
    ew1 = np.asarray(ew1, np.float32)
    eb1 = np.asarray(eb1, np.float32)
    ew2 = np.asarray(ew2, np.float32)
    eb2 = np.asarray(eb2, np.float32)

    # ew1/ew2 pretiled to [128, kt*cols] so the DMA is one contiguous
    # run per partition (row c = kt*128 + p -> partition p, block kt)
    ew1t = np.ascontiguousarray(
        ew1.reshape(20, 128, H).transpose(1, 0, 2).reshape(128, 20 * H)
        .astype(BF16NP)
    )
    ew2t = np.ascontiguousarray(
        ew2.reshape(4, 128, E).transpose(1, 0, 2).reshape(128, 4 * E)
        .astype(BF16NP)
    )
    shared = {
        "eb1r": np.ascontiguousarray(eb1.reshape(1, H).astype(BF16NP)),
        "ew1t": ew1t,
        "ew2t": ew2t,
    }
    packB = np.concatenate(
        [cw1, cw2, np.zeros((2 * A, A), np.float32)], axis=1
    ).astype(BF16NP)
    in_maps = []
    ones64 = np.ones((A, N), np.float32)
    zeros64 = np.zeros((A, N), np.float32)
    for b in range(B):
        afvT = np.ascontiguousarray(afv[b].T)  # [64, 128]
        m = dict(shared)
        # gaT: [P, (r', n)] r'-major columns
        m["gaT"] = np.ascontiguousarray(
            ga[b].transpose(1, 0, 2).reshape(M2, P).T.astype(BF16NP)
        )
        # grT: [m, (r, n)] r-major so per-r rhs slices are contiguous
        m["grT"] = np.ascontiguousarray(
            gr[b].transpose(2, 1, 0).reshape(N, G2).astype(BF16NP)
        )
        pb = packB.copy()
        pb[:, 160:224] = afv[b].astype(BF16NP)
        m["packB"] = pb
        pf = np.empty((128, 419), np.float32)
        pf[:, 0:1] = cb1.reshape(2 * A, 1)
        pf[:, 1:33] = np.broadcast_to(cb2, (2 * A, D))
        pf[:, 33:161] = np.concatenate([afvT, afvT], axis=0)
        pf[:, 161:289] = np.concatenate([ones64, afvT], axis=0)
        pf[:, 289:417] = np.concatenate([afvT, zeros64], axis=0)
        pf[:, 417:419] = eb2.reshape(2, 128).T
        m["packF"] = pf
        in_maps.append(m)
    return in_maps


def run(inputs: dict, trace: bool = False):
    """Returns ((aef, afv), exec_time_ns_or_None)."""
    nc = _get_nc()
    in_maps = _prep_in_maps(**inputs)
    res = run_bass_kernel_spmd(nc, in_maps, core_ids=list(range(B)), trace=trace)
    aef = np.stack(
        [np.ascontiguousarray(res.results[b]["out"].T) for b in range(B)], axis=0
    )
    afv = np.asarray(inputs["afv"], np.float32)
    return (aef, afv), res.exec_time_ns


def kernel(**inputs) -> np.ndarray:
    (aef, afv), _ = run(inputs, trace=False)
    return aef, afv
